# revision 19
# baseline (speedup 1.0000x reference)
"""Multi-head attention (B=2, S=2048, d_model=768, H=12) on 8 TRN2 NeuronCores.

Sharding: 2-way data parallel over batch x 4-way tensor parallel over heads
(3 heads / 192-wide d_model slice per core). Host compacts masked keys away
(gather of unmasked key/value rows), pads to a 128 multiple, and passes a 0/1
validity vector; softmax needs no mask handling on device (pad keys get V=0
and a 0 in the denominator ones-column).

v3 schedule (per core):
  - All inputs host-preformatted into a few large contiguous [128, X]
    transfers (weights [p, kt*DQ+j], xk/xv [p, kt*KP+j], xq chunked
    [p, (c*NKT+kt)*CW+j], consts packed into one tile): each DMA_DIRECT2D
    costs ~0.6us of serialized sync-engine issue, so DMA count matters more
    than bytes for the startup critical path.
  - All PSUM->SBUF evictions on DVE (tensor_scalar_add with bias); ACT does
    exp only - it is the attention pacer at (N+352)/1.2 ns per instruction.
  - Attention per 512-query chunk: loop A does heads 0/1 (scores paired into
    disjoint C=64 PE row groups, shared exp over [128,1024] PSUM, PV into
    ctx0/ctx1), then h2 runs as a deferred loop B (scores unpaired; the loop
    is ACT-bound so the extra PE time hides). PSUM = 8 banks: scores 2x2,
    ctx 2x1, po/ctx2/qproj/vproj shared pool 2x1.
  - Emission software-pipelined: scores(t+1) enter the PE queue before PV(t);
    O-projection of chunk c-1 and Q-projection of chunk c+1 interleave into
    chunk c's loop as PE filler; chunk 0 runs a deeper pipeline so scores/exp
    stream while XV is still landing.
  - HW rules learned the hard way: tensor_tensor with two SBUF inputs needs
    EQUAL base partitions (outputs may be offset); never partition-shift via
    tensor_scalar writes; stage softmax denominators to SBUF before recip.
  - Output partials written bf16 (halves out DMA + evict cost), host sums.
"""

import math

import numpy as np

B = 2
S = 2048
DM = 768
H = 12
DH = 64
G = 4              # head-group (tensor-parallel) degree
HPG = H // G       # heads per core
DQ = HPG * DH      # 192 d_model slice per core
NCORES = 8
P = 128
NKT = DM // P      # 6 contraction tiles for projections
CW = 512           # query chunk width
NCH = S // CW      # 4 query chunks

_prog_cache = {}


def _chunks(total, step):
    out = []
    o = 0
    while o < total:
        w = min(step, total - o)
        out.append((o, w))
        o += w
    return out


def _build_nc(KP):
    import concourse.bass as bass
    import concourse.mybir as mybir
    import concourse.tile as tile
    from concourse import bacc

    F32 = mybir.dt.float32
    BF = mybir.dt.bfloat16
    AFT = mybir.ActivationFunctionType

    T = KP // P            # key tiles
    KCH = _chunks(KP, 512)

    nc = bacc.Bacc(None, target_bir_lowering=False)
    # all inputs host-preformatted into few large contiguous transfers
    # (each DMA_DIRECT2D costs ~0.6us of serialized sync-engine issue)
    xqf = nc.declare_dram_parameter("xqf", [P, NCH * NKT * CW], BF, isOutput=False)
    xkf = nc.declare_dram_parameter("xkf", [P, NKT * KP], BF, isOutput=False)
    xvf = nc.declare_dram_parameter("xvf", [P, NKT * KP], BF, isOutput=False)
    wq = nc.declare_dram_parameter("wq", [P, NKT * DQ], BF, isOutput=False)
    wk = nc.declare_dram_parameter("wk", [P, NKT * DQ], BF, isOutput=False)
    wv = nc.declare_dram_parameter("wv", [P, NKT * DQ], BF, isOutput=False)
    wof = nc.declare_dram_parameter("wof", [P, 2 * DM], BF, isOutput=False)
    NCONS = 4 + T + DQ
    cons = nc.declare_dram_parameter("cons", [P, NCONS], F32, isOutput=False)
    out = nc.declare_dram_parameter("out", [S, DM], BF, isOutput=True)

    scale = 1.0 / math.sqrt(DH)

    with tile.TileContext(nc) as tc:
        with (
            tc.tile_pool(name="persist", bufs=1) as persist,
            tc.tile_pool(name="es", bufs=6) as espool,
            tc.tile_pool(name="norm", bufs=2) as norm,
            tc.tile_pool(name="osb", bufs=3) as osb,
            tc.tile_pool(name="sp_ps", bufs=2, space="PSUM") as sp_ps,
            tc.tile_pool(name="ctx_ps", bufs=2, space="PSUM") as ctx_ps,
            tc.tile_pool(name="po_ps", bufs=2, space="PSUM") as po_ps,
        ):
            # ---- persistent tiles ----
            WQ = persist.tile([P, NKT * DQ], BF, tag="WQ")
            WK = persist.tile([P, NKT * DQ], BF, tag="WK")
            WV = persist.tile([P, NKT * DQ], BF, tag="WV")
            WOF = persist.tile([P, 2 * DM], BF, tag="WOF")
            WO0 = WOF[:, 0:DM]           # wo rows 0:128 (h0,h1)
            WO2 = WOF[0:DH, DM:2 * DM]   # wo rows 128:192 (h2)
            CONS = persist.tile([P, NCONS], F32, tag="CONS")
            BQ0 = CONS[:, 0:1]
            BQ1 = CONS[0:DH, 1:2]
            BK0 = CONS[:, 2:3]
            BK1 = CONS[0:DH, 3:4]
            VM = CONS[:, 4:4 + T]
            BV = CONS[:, 4 + T:4 + T + DQ]
            QT0 = persist.tile([P, S], BF, tag="QT0")    # heads 0,1
            QT1 = persist.tile([P, S], BF, tag="QT1")    # head 2 (+dup rows)
            KT0 = persist.tile([P, KP], BF, tag="KT0")
            KT1 = persist.tile([P, KP], BF, tag="KT1")   # rows 0:64 + DMA dup
            # V blocks per (t, head): [V_h(64) | valid(1) | zero(63)]
            VP = persist.tile([P, T, HPG * P], BF, tag="VP")
            CTX01 = persist.tile([P, S], BF, tag="CTX01")  # h0 rows 0:64, h1 64:128
            CTX2 = persist.tile([DH, S], BF, tag="CTX2")

            XKb0 = persist.tile([P, NKT // 2, KP], BF, tag="XKb0")
            XKb1 = persist.tile([P, NKT - NKT // 2, KP], BF, tag="XKb1")
            XVb = persist.tile([P, NKT, KP], BF, tag="XVb")
            XQb = persist.tile([P, NCH, NKT, CW], BF, tag="XQb")

            # ---- DMA issue, priority order ----
            nc.sync.dma_start(out=WK, in_=wk[:, :])
            nc.sync.dma_start(
                out=XKb0,
                in_=xkf[:, 0:(NKT // 2) * KP].rearrange("p (kt j) -> p kt j", j=KP),
            )
            nc.sync.dma_start(
                out=XKb1,
                in_=xkf[:, (NKT // 2) * KP:].rearrange("p (kt j) -> p kt j", j=KP),
            )
            nc.sync.dma_start(out=WQ, in_=wq[:, :])
            nc.sync.dma_start(
                out=XQb[:, 0, :, :],
                in_=xqf[:, 0:NKT * CW].rearrange("p (kt j) -> p kt j", j=CW),
            )
            nc.sync.dma_start(out=CONS, in_=cons[:, :])
            nc.sync.dma_start(out=WV, in_=wv[:, :])
            nc.sync.dma_start(
                out=XVb, in_=xvf[:, :].rearrange("p (kt j) -> p kt j", j=KP)
            )
            for c in range(1, NCH):
                nc.sync.dma_start(
                    out=XQb[:, c, :, :],
                    in_=xqf[:, c * NKT * CW:(c + 1) * NKT * CW].rearrange(
                        "p (kt j) -> p kt j", j=CW
                    ),
                )
            nc.sync.dma_start(out=WOF, in_=wof[:, :])
            nc.vector.memset(VP, 0.0)
            XK = [XKb0[:, kt, :] for kt in range(NKT // 2)] + [
                XKb1[:, kt, :] for kt in range(NKT - NKT // 2)]
            XV = [XVb[:, kt, :] for kt in range(NKT)]

            # ---- phase A: K projection (DVE evicts; ACT stays free for exp) ----
            for ci, (c0, cwk) in enumerate(KCH):
                kp0 = ctx_ps.tile([P, CW], F32, tag="ctx", name=f"kp0_{ci}")
                for kt in range(NKT):
                    nc.tensor.matmul(
                        kp0[:, 0:cwk],
                        lhsT=WK[:, kt * DQ:kt * DQ + P],
                        rhs=XK[kt][:, c0:c0 + cwk],
                        start=(kt == 0), stop=(kt == NKT - 1),
                    )
                nc.vector.tensor_scalar_add(
                    KT0[:, c0:c0 + cwk], kp0[:, 0:cwk], BK0
                )
                kp1 = ctx_ps.tile([P, CW], F32, tag="ctx", name=f"kp1_{ci}")
                for kt in range(NKT):
                    nc.tensor.matmul(
                        kp1[0:DH, 0:cwk],
                        lhsT=WK[:, kt * DQ + P:kt * DQ + DQ],
                        rhs=XK[kt][:, c0:c0 + cwk],
                        start=(kt == 0), stop=(kt == NKT - 1),
                    )
                nc.vector.tensor_scalar_add(
                    KT1[0:DH, c0:c0 + cwk], kp1[0:DH, 0:cwk], BK1
                )

            nc.sync.dma_start(out=KT1[DH:P, :], in_=KT1[0:DH, :])

            def qproj(c0, which):
                """which 0 -> heads 0/1 slice, 1 -> head 2 slice (+dup)."""
                qp = po_ps.tile([P, CW], F32, tag="po", name=f"qp{which}_{c0}")
                if which == 0:
                    for kt in range(NKT):
                        nc.tensor.matmul(
                            qp[:, :],
                            lhsT=WQ[:, kt * DQ:kt * DQ + P],
                            rhs=XQb[:, c0 // CW, kt, :],
                            start=(kt == 0), stop=(kt == NKT - 1),
                        )
                    nc.vector.tensor_scalar_add(QT0[:, c0:c0 + CW], qp[:, :], BQ0)
                else:
                    for kt in range(NKT):
                        nc.tensor.matmul(
                            qp[0:DH, :],
                            lhsT=WQ[:, kt * DQ + P:kt * DQ + DQ],
                            rhs=XQb[:, c0 // CW, kt, :],
                            start=(kt == 0), stop=(kt == NKT - 1),
                        )
                    nc.vector.tensor_scalar_add(
                        QT1[0:DH, c0:c0 + CW], qp[0:DH, :], BQ1
                    )
                    nc.sync.dma_start(
                        out=QT1[DH:P, c0:c0 + CW], in_=QT1[0:DH, c0:c0 + CW]
                    )

            def vproj(t):
                vp = po_ps.tile([P, CW], F32, tag="po", name=f"vp{t}")
                for kt in range(NKT):
                    nc.tensor.matmul(
                        vp[:, 0:DQ],
                        lhsT=XV[kt][:, t * P:(t + 1) * P],
                        rhs=WV[:, kt * DQ:(kt + 1) * DQ],
                        start=(kt == 0), stop=(kt == NKT - 1),
                    )
                vv = VP[:, t, :].rearrange("p (h c) -> p h c", c=P)
                nc.vector.tensor_add(
                    vv[:, :, 0:DH],
                    vp[:, 0:DQ].rearrange("p (h d) -> p h d", d=DH),
                    BV[:, :].rearrange("p (h d) -> p h d", d=DH),
                )
                nc.vector.tensor_scalar_mul(
                    vv[:, :, 0:DH], vv[:, :, 0:DH], VM[:, t:t + 1]
                )
                nc.vector.tensor_copy(
                    vv[:, :, DH:DH + 1],
                    VM[:, t:t + 1].to_broadcast([P, HPG, 1]),
                )

            def oproj(m):
                lhA = CTX01[:, m * P:(m + 1) * P]
                lhB = CTX2[:, m * P:(m + 1) * P]
                po_a = po_ps.tile([P, CW], F32, tag="po", name=f"poa{m}")
                nc.tensor.matmul(po_a[:, :], lhsT=lhA, rhs=WO0[:, 0:CW],
                                 start=True, stop=False)
                nc.tensor.matmul(po_a[:, :], lhsT=lhB, rhs=WO2[:, 0:CW],
                                 start=False, stop=True)
                po_b = po_ps.tile([P, CW], F32, tag="po", name=f"pob{m}")
                nc.tensor.matmul(po_b[:, 0:DM - CW], lhsT=lhA, rhs=WO0[:, CW:DM],
                                 start=True, stop=False)
                nc.tensor.matmul(po_b[:, 0:DM - CW], lhsT=lhB, rhs=WO2[:, CW:DM],
                                 start=False, stop=True)
                psb = osb.tile([P, DM], BF, tag="posb", name=f"psb{m}")
                nc.vector.tensor_copy(psb[:, 0:CW], po_a[:, :])
                nc.vector.tensor_copy(psb[:, CW:DM], po_b[:, 0:DM - CW])
                nc.sync.dma_start(out=out[m * P:(m + 1) * P, :], in_=psb)

            # ---- phase A tail: Q projection for chunk 0 ----
            qproj(0, 0)
            qproj(0, 1)

            # ---- attention chunks ----
            def scores01(ci, t):
                sp = sp_ps.tile([P, 2 * CW], F32, tag="sp", name=f"sp{ci}_{t}")
                c0 = ci * CW
                nc.tensor.matmul(
                    sp[:, 0:CW],
                    lhsT=KT0[0:DH, t * P:(t + 1) * P],
                    rhs=QT0[0:DH, c0:c0 + CW],
                    start=True, stop=True,
                )
                nc.tensor.matmul(
                    sp[:, CW:2 * CW],
                    lhsT=KT0[DH:P, t * P:(t + 1) * P],
                    rhs=QT0[DH:P, c0:c0 + CW],
                    start=True, stop=True,
                )
                return sp

            for ci in range(NCH):
                c0 = ci * CW
                ctx0 = ctx_ps.tile([P, CW], F32, tag="ctx", name=f"c0_{ci}")
                ctx1 = ctx_ps.tile([P, CW], F32, tag="ctx", name=f"c1_{ci}")

                def pv01(t, es):
                    nc.tensor.matmul(
                        ctx0[:, :],
                        lhsT=VP[:, t, 0:P],
                        rhs=es[:, 0:CW],
                        start=(t == 0), stop=(t == T - 1),
                    )
                    nc.tensor.matmul(
                        ctx1[:, :],
                        lhsT=VP[:, t, P:2 * P],
                        rhs=es[:, CW:2 * CW],
                        start=(t == 0), stop=(t == T - 1),
                    )

                def do_exp(sp, t):
                    es = espool.tile([P, 2 * CW], BF, tag="es", name=f"es{ci}_{t}")
                    nc.scalar.activation(es, sp, AFT.Exp, bias=0.0, scale=scale)
                    return es

                extras = {}
                if ci == 0:
                    # Deep pipeline: XV lands well after XQ, so scores/exp run
                    # ahead while vproj(t)/pv(t-1) trail one step behind.
                    sps = {0: scores01(ci, 0), 1: scores01(ci, 1)}
                    ess = {0: do_exp(sps.pop(0), 0)}
                    for t in range(T):
                        if t + 2 < T:
                            sps[t + 2] = scores01(ci, t + 2)
                        if t + 1 < T:
                            ess[t + 1] = do_exp(sps.pop(t + 1), t + 1)
                        vproj(t)
                        if t >= 1:
                            pv01(t - 1, ess.pop(t - 1))
                    pv01(T - 1, ess.pop(T - 1))
                else:
                    mlist = [(ci - 1) * 4 + i for i in range(4)]
                    for sl, m in zip([2, 4, 6, 8], mlist):
                        extras[sl] = (lambda mm: (lambda: oproj(mm)))(m)
                    if ci < NCH - 1:
                        nc0 = (ci + 1) * CW
                        extras[3] = (lambda cc: (lambda: qproj(cc, 0)))(nc0)
                        extras[5] = (lambda cc: (lambda: qproj(cc, 1)))(nc0)
                    sp_cur = scores01(ci, 0)
                    for t in range(T):
                        sp_next = scores01(ci, t + 1) if t + 1 < T else None
                        es = do_exp(sp_cur, t)
                        if t + 2 in extras:
                            extras.pop(t + 2)()
                        pv01(t, es)
                        sp_cur = sp_next

                # free ctx0/ctx1: evict numerators + denominators (SBUF), then
                # reciprocals from SBUF. All tensor-tensor inputs share base
                # partition 0 (HW requires equal SB input bases); only outputs
                # may be partition-offset (baseline-proven).
                cs0 = norm.tile([DH, CW], F32, tag="cs0", name=f"cs0_{ci}")
                cs1 = norm.tile([DH, CW], F32, tag="cs1", name=f"cs1_{ci}")
                nc.vector.tensor_copy(cs0, ctx0[0:DH, :])
                nc.vector.tensor_copy(cs1, ctx1[0:DH, :])
                dn = norm.tile([1, 3 * CW], F32, tag="dn", name=f"dn{ci}")
                nc.vector.tensor_copy(dn[:, 0:CW], ctx0[DH:DH + 1, :])
                nc.vector.tensor_copy(dn[:, CW:2 * CW], ctx1[DH:DH + 1, :])
                rc = norm.tile([1, 3 * CW], F32, tag="rc", name=f"rc{ci}")
                nc.vector.reciprocal_approx_fast(rc[:, 0:2 * CW], dn[:, 0:2 * CW])

                # loop B: head 2, deferred; paired scores via duplicated KT1/QT1
                if ci == 0:
                    ctx2 = ctx_ps.tile([P, CW], F32, tag="ctx", name=f"c2_{ci}")
                else:
                    ctx2 = po_ps.tile([P, CW], F32, tag="po", name=f"c2_{ci}")
                for tg0 in range(0, T, 2):
                    tl = [tg0, tg0 + 1] if tg0 + 1 < T else [tg0]
                    ln = len(tl)
                    sp2 = sp_ps.tile([P, 2 * CW], F32, tag="sp", name=f"sp2_{ci}_{tg0}")
                    for i, t in enumerate(tl):
                        r0 = (i % 2) * DH
                        nc.tensor.matmul(
                            sp2[:, i * CW:(i + 1) * CW],
                            lhsT=KT1[r0:r0 + DH, t * P:(t + 1) * P],
                            rhs=QT1[r0:r0 + DH, c0:c0 + CW],
                            start=True, stop=True,
                        )
                    es2 = espool.tile([P, 2 * CW], BF, tag="es", name=f"es2_{ci}_{tg0}")
                    nc.scalar.activation(
                        es2[:, 0:ln * CW], sp2[:, 0:ln * CW],
                        AFT.Exp, bias=0.0, scale=scale,
                    )
                    if ci == 0 and NCH > 1 and tg0 == 0:
                        qproj(CW, 0)
                    if ci == 0 and NCH > 1 and tg0 == 2:
                        qproj(CW, 1)
                    if tg0 + 2 >= T and extras:
                        for k in sorted(extras):
                            extras.pop(k)()
                    for i, t in enumerate(tl):
                        nc.tensor.matmul(
                            ctx2[:, :],
                            lhsT=VP[:, t, 2 * P:3 * P],
                            rhs=es2[:, i * CW:(i + 1) * CW],
                            start=(t == 0), stop=(t == T - 1),
                        )

                cs2 = norm.tile([DH, CW], F32, tag="cs2", name=f"cs2_{ci}")
                nc.vector.tensor_copy(cs2, ctx2[0:DH, :])
                nc.vector.tensor_copy(dn[:, 2 * CW:3 * CW], ctx2[DH:DH + 1, :])
                nc.vector.reciprocal_approx_fast(rc[:, 2 * CW:3 * CW], dn[:, 2 * CW:3 * CW])
                bc0 = norm.tile([DH, CW], F32, tag="bc0", name=f"bc0_{ci}")
                bc1 = norm.tile([DH, CW], F32, tag="bc1", name=f"bc1_{ci}")
                bc2 = norm.tile([DH, CW], F32, tag="bc2", name=f"bc2_{ci}")
                nc.gpsimd.partition_broadcast(bc0, rc[:, 0:CW])
                nc.vector.tensor_mul(CTX01[0:DH, c0:c0 + CW], cs0, bc0)
                nc.gpsimd.partition_broadcast(bc1, rc[:, CW:2 * CW])
                nc.vector.tensor_mul(CTX01[DH:P, c0:c0 + CW], cs1, bc1)
                nc.gpsimd.partition_broadcast(bc2, rc[:, 2 * CW:3 * CW])
                nc.vector.tensor_mul(CTX2[:, c0:c0 + CW], cs2, bc2)

            # tail: O-projection of the last chunk
            for m in range((NCH - 1) * 4, NCH * 4):
                oproj(m)
    nc.compile()
    return nc


def _get_prog(KP):
    if KP not in _prog_cache:
        _prog_cache[KP] = _build_nc(KP)
    return _prog_cache[KP]


def _fmt_w(w):
    # [768, 192] -> [128, 6*192]: row kt*128+p, col j  ->  [p, kt*192+j]
    import ml_dtypes
    return np.ascontiguousarray(
        w.reshape(NKT, P, DQ).transpose(1, 0, 2).reshape(P, NKT * DQ)
    ).astype(ml_dtypes.bfloat16)


def _run(inputs, trace=False):
    import ml_dtypes
    from concourse.bass_utils import run_bass_kernel_spmd

    BF = ml_dtypes.bfloat16

    query = np.asarray(inputs["query"], dtype=np.float32)
    key = np.asarray(inputs["key"], dtype=np.float32)
    value = np.asarray(inputs["value"], dtype=np.float32)
    mask = np.asarray(inputs["mask"])
    Wq = np.asarray(inputs["Wq"], dtype=np.float32)
    bq = np.asarray(inputs["bq"], dtype=np.float32)
    Wk = np.asarray(inputs["Wk"], dtype=np.float32)
    bk = np.asarray(inputs["bk"], dtype=np.float32)
    Wv = np.asarray(inputs["Wv"], dtype=np.float32)
    bv = np.asarray(inputs["bv"], dtype=np.float32)
    Wo = np.asarray(inputs["Wo"], dtype=np.float32)
    bo = np.asarray(inputs["bo"], dtype=np.float32)

    idx = [np.nonzero(mask[b, 0, 0] != 0)[0] for b in range(B)]
    keff = [len(i) for i in idx]
    KP = max(P, ((max(keff) + P - 1) // P) * P)
    T = KP // P

    nc = _get_prog(KP)

    per_batch = {}
    for b in range(B):
        xqT = np.ascontiguousarray(query[b].T).astype(BF)
        xkT = np.zeros((DM, KP), dtype=BF)
        xkT[:, :keff[b]] = key[b][idx[b]].T.astype(BF)
        xvT = np.zeros((DM, KP), dtype=BF)
        xvT[:, :keff[b]] = value[b][idx[b]].T.astype(BF)
        vmf = np.zeros((KP,), dtype=np.float32)
        vmf[:keff[b]] = 1.0
        vm2 = np.ascontiguousarray(vmf.reshape(T, P).T)  # [128, T]
        # packed layouts: one large contiguous DMA each
        xqfm = np.ascontiguousarray(
            xqT.reshape(NKT, P, NCH, CW).transpose(1, 2, 0, 3).reshape(P, -1))
        xkfm = np.ascontiguousarray(
            xkT.reshape(NKT, P, KP).transpose(1, 0, 2).reshape(P, -1))
        xvfm = np.ascontiguousarray(
            xvT.reshape(NKT, P, KP).transpose(1, 0, 2).reshape(P, -1))
        per_batch[b] = (xqfm, xkfm, xvfm, vm2)

    in_maps = []
    for core in range(NCORES):
        b, g = core // G, core % G
        xqfm, xkfm, xvfm, vm2 = per_batch[b]
        sl = slice(g * DQ, (g + 1) * DQ)
        wo_sl = Wo[sl, :]
        wofm = np.zeros((P, 2 * DM), dtype=BF)
        wofm[0:P, 0:DM] = wo_sl[0:P, :].astype(BF)
        wofm[0:DH, DM:2 * DM] = wo_sl[P:DQ, :].astype(BF)
        consm = np.zeros((P, 4 + T + DQ), dtype=np.float32)
        consm[0:P, 0] = bq[sl][0:P]
        consm[0:DH, 1] = bq[sl][P:DQ]
        consm[0:P, 2] = bk[sl][0:P]
        consm[0:DH, 3] = bk[sl][P:DQ]
        consm[:, 4:4 + T] = vm2
        consm[:, 4 + T:4 + T + DQ] = bv[sl].reshape(1, DQ)
        in_maps.append({
            "xqf": xqfm,
            "xkf": xkfm,
            "xvf": xvfm,
            "wq": _fmt_w(Wq[:, sl]),
            "wk": _fmt_w(Wk[:, sl]),
            "wv": _fmt_w(Wv[:, sl]),
            "wof": wofm,
            "cons": consm,
        })

    res = run_bass_kernel_spmd(nc, in_maps, list(range(NCORES)), trace=trace)

    outp = np.zeros((B, S, DM), dtype=np.float32)
    for core in range(NCORES):
        outp[core // G] += res.results[core]["out"].astype(np.float32)
    outp += bo.reshape(1, 1, DM)
    return outp, res


def kernel(**inputs) -> np.ndarray:
    out, _ = _run(inputs, trace=False)
    return out


if __name__ == "__main__":
    nc = _build_nc(1152)
    print("build OK")


# revision 20
# speedup vs baseline: 1.0122x; 1.0122x over previous
"""Multi-head attention (B=2, S=2048, d_model=768, H=12) on 8 TRN2 NeuronCores.

Sharding: 2-way data parallel over batch x 4-way tensor parallel over heads
(3 heads / 192-wide d_model slice per core). Host compacts masked keys away
(gather of unmasked key/value rows), pads to a 128 multiple, and passes a 0/1
validity vector; softmax needs no mask handling on device (pad keys get V=0
and a 0 in the denominator ones-column).

v3 schedule (per core):
  - All inputs host-preformatted into a few large contiguous [128, X]
    transfers (weights [p, kt*DQ+j], xk/xv [p, kt*KP+j], xq chunked
    [p, (c*NKT+kt)*CW+j], consts packed into one tile): each DMA_DIRECT2D
    costs ~0.6us of serialized sync-engine issue, so DMA count matters more
    than bytes for the startup critical path.
  - All PSUM->SBUF evictions on DVE (tensor_scalar_add with bias); ACT does
    exp only - it is the attention pacer at (N+352)/1.2 ns per instruction.
  - Attention per 512-query chunk: loop A does heads 0/1 (scores paired into
    disjoint C=64 PE row groups, shared exp over [128,1024] PSUM, PV into
    ctx0/ctx1), then h2 runs as a deferred loop B (scores unpaired; the loop
    is ACT-bound so the extra PE time hides). PSUM = 8 banks: scores 2x2,
    ctx 2x1, po/ctx2/qproj/vproj shared pool 2x1.
  - Emission software-pipelined: scores(t+1) enter the PE queue before PV(t);
    O-projection of chunk c-1 and Q-projection of chunk c+1 interleave into
    chunk c's loop as PE filler; chunk 0 runs a deeper pipeline so scores/exp
    stream while XV is still landing.
  - HW rules learned the hard way: tensor_tensor with two SBUF inputs needs
    EQUAL base partitions (outputs may be offset); never partition-shift via
    tensor_scalar writes; stage softmax denominators to SBUF before recip.
  - Output partials written bf16 (halves out DMA + evict cost), host sums.
"""

import math

import numpy as np

B = 2
S = 2048
DM = 768
H = 12
DH = 64
G = 4              # head-group (tensor-parallel) degree
HPG = H // G       # heads per core
DQ = HPG * DH      # 192 d_model slice per core
NCORES = 8
P = 128
NKT = DM // P      # 6 contraction tiles for projections
CW = 512           # query chunk width
NCH = S // CW      # 4 query chunks

_prog_cache = {}


def _chunks(total, step):
    out = []
    o = 0
    while o < total:
        w = min(step, total - o)
        out.append((o, w))
        o += w
    return out


def _build_nc(KP):
    import concourse.bass as bass
    import concourse.mybir as mybir
    import concourse.tile as tile
    from concourse import bacc

    F32 = mybir.dt.float32
    BF = mybir.dt.bfloat16
    AFT = mybir.ActivationFunctionType

    T = KP // P            # key tiles
    KCH = _chunks(KP, 512)

    nc = bacc.Bacc(None, target_bir_lowering=False)
    # all inputs host-preformatted into few large contiguous transfers
    # (each DMA_DIRECT2D costs ~0.6us of serialized sync-engine issue)
    xqf = nc.declare_dram_parameter("xqf", [P, NCH * NKT * CW], BF, isOutput=False)
    xkf = nc.declare_dram_parameter("xkf", [P, NKT * KP], BF, isOutput=False)
    xvf = nc.declare_dram_parameter("xvf", [P, NKT * KP], BF, isOutput=False)
    wq = nc.declare_dram_parameter("wq", [P, NKT * DQ], BF, isOutput=False)
    wk = nc.declare_dram_parameter("wk", [P, NKT * DQ], BF, isOutput=False)
    wv = nc.declare_dram_parameter("wv", [P, NKT * DQ], BF, isOutput=False)
    wof = nc.declare_dram_parameter("wof", [P, 2 * DM], BF, isOutput=False)
    NCONS = 4 + T + DQ
    cons = nc.declare_dram_parameter("cons", [P, NCONS], F32, isOutput=False)
    out = nc.declare_dram_parameter("out", [S, DM], BF, isOutput=True)

    scale = 1.0 / math.sqrt(DH)

    with tile.TileContext(nc) as tc:
        with (
            tc.tile_pool(name="persist", bufs=1) as persist,
            tc.tile_pool(name="es", bufs=6) as espool,
            tc.tile_pool(name="norm", bufs=2) as norm,
            tc.tile_pool(name="osb", bufs=3) as osb,
            tc.tile_pool(name="sp_ps", bufs=2, space="PSUM") as sp_ps,
            tc.tile_pool(name="ctx_ps", bufs=2, space="PSUM") as ctx_ps,
            tc.tile_pool(name="po_ps", bufs=2, space="PSUM") as po_ps,
        ):
            # ---- persistent tiles ----
            WQ = persist.tile([P, NKT * DQ], BF, tag="WQ")
            WK = persist.tile([P, NKT * DQ], BF, tag="WK")
            WV = persist.tile([P, NKT * DQ], BF, tag="WV")
            WOF = persist.tile([P, 2 * DM], BF, tag="WOF")
            WO0 = WOF[:, 0:DM]           # wo rows 0:128 (h0,h1)
            WO2 = WOF[0:DH, DM:2 * DM]   # wo rows 128:192 (h2)
            CONS = persist.tile([P, NCONS], F32, tag="CONS")
            BQ0 = CONS[:, 0:1]
            BQ1 = CONS[0:DH, 1:2]
            BK0 = CONS[:, 2:3]
            BK1 = CONS[0:DH, 3:4]
            VM = CONS[:, 4:4 + T]
            BV = CONS[:, 4 + T:4 + T + DQ]
            QT0 = persist.tile([P, S], BF, tag="QT0")    # heads 0,1
            QT1 = persist.tile([P, S], BF, tag="QT1")    # head 2 (+dup rows)
            KT0 = persist.tile([P, KP], BF, tag="KT0")
            KT1 = persist.tile([P, KP], BF, tag="KT1")   # rows 0:64 + DMA dup
            # V blocks per (t, head): [V_h(64) | valid(1) | zero(63)]
            VP = persist.tile([P, T, HPG * P], BF, tag="VP")
            CTX01 = persist.tile([P, S], BF, tag="CTX01")  # h0 rows 0:64, h1 64:128
            CTX2 = persist.tile([DH, S], BF, tag="CTX2")

            XKb0 = persist.tile([P, NKT // 2, KP], BF, tag="XKb0")
            XKb1 = persist.tile([P, NKT - NKT // 2, KP], BF, tag="XKb1")
            XVb = persist.tile([P, NKT, KP], BF, tag="XVb")
            XQb = persist.tile([P, NCH, NKT, CW], BF, tag="XQb")

            # ---- DMA issue, priority order ----
            nc.sync.dma_start(out=WK, in_=wk[:, :])
            nc.sync.dma_start(
                out=XKb0,
                in_=xkf[:, 0:(NKT // 2) * KP].rearrange("p (kt j) -> p kt j", j=KP),
            )
            nc.sync.dma_start(
                out=XKb1,
                in_=xkf[:, (NKT // 2) * KP:].rearrange("p (kt j) -> p kt j", j=KP),
            )
            nc.sync.dma_start(out=WQ, in_=wq[:, :])
            nc.sync.dma_start(
                out=XQb[:, 0, :, :],
                in_=xqf[:, 0:NKT * CW].rearrange("p (kt j) -> p kt j", j=CW),
            )
            nc.sync.dma_start(out=CONS, in_=cons[:, :])
            nc.sync.dma_start(out=WV, in_=wv[:, :])
            nc.sync.dma_start(
                out=XVb, in_=xvf[:, :].rearrange("p (kt j) -> p kt j", j=KP)
            )
            for c in range(1, NCH):
                nc.sync.dma_start(
                    out=XQb[:, c, :, :],
                    in_=xqf[:, c * NKT * CW:(c + 1) * NKT * CW].rearrange(
                        "p (kt j) -> p kt j", j=CW
                    ),
                )
            nc.sync.dma_start(out=WOF, in_=wof[:, :])
            nc.vector.memset(VP, 0.0)
            XK = [XKb0[:, kt, :] for kt in range(NKT // 2)] + [
                XKb1[:, kt, :] for kt in range(NKT - NKT // 2)]
            XV = [XVb[:, kt, :] for kt in range(NKT)]

            # ---- phase A: K projection (DVE evicts; ACT stays free for exp) ----
            for ci, (c0, cwk) in enumerate(KCH):
                kp0 = ctx_ps.tile([P, CW], F32, tag="ctx", name=f"kp0_{ci}")
                for kt in range(NKT):
                    nc.tensor.matmul(
                        kp0[:, 0:cwk],
                        lhsT=WK[:, kt * DQ:kt * DQ + P],
                        rhs=XK[kt][:, c0:c0 + cwk],
                        start=(kt == 0), stop=(kt == NKT - 1),
                    )
                nc.vector.tensor_scalar_add(
                    KT0[:, c0:c0 + cwk], kp0[:, 0:cwk], BK0
                )
                kp1 = ctx_ps.tile([P, CW], F32, tag="ctx", name=f"kp1_{ci}")
                for kt in range(NKT):
                    nc.tensor.matmul(
                        kp1[0:DH, 0:cwk],
                        lhsT=WK[:, kt * DQ + P:kt * DQ + DQ],
                        rhs=XK[kt][:, c0:c0 + cwk],
                        start=(kt == 0), stop=(kt == NKT - 1),
                    )
                nc.vector.tensor_scalar_add(
                    KT1[0:DH, c0:c0 + cwk], kp1[0:DH, 0:cwk], BK1
                )

            nc.sync.dma_start(out=KT1[DH:P, :], in_=KT1[0:DH, :])

            def qproj(c0, which):
                """which 0 -> heads 0/1 slice, 1 -> head 2 slice (+dup)."""
                qp = po_ps.tile([P, CW], F32, tag="po", name=f"qp{which}_{c0}")
                if which == 0:
                    for kt in range(NKT):
                        nc.tensor.matmul(
                            qp[:, :],
                            lhsT=WQ[:, kt * DQ:kt * DQ + P],
                            rhs=XQb[:, c0 // CW, kt, :],
                            start=(kt == 0), stop=(kt == NKT - 1),
                        )
                    nc.vector.tensor_scalar_add(QT0[:, c0:c0 + CW], qp[:, :], BQ0)
                else:
                    for kt in range(NKT):
                        nc.tensor.matmul(
                            qp[0:DH, :],
                            lhsT=WQ[:, kt * DQ + P:kt * DQ + DQ],
                            rhs=XQb[:, c0 // CW, kt, :],
                            start=(kt == 0), stop=(kt == NKT - 1),
                        )
                    nc.vector.tensor_scalar_add(
                        QT1[0:DH, c0:c0 + CW], qp[0:DH, :], BQ1
                    )
                    nc.sync.dma_start(
                        out=QT1[DH:P, c0:c0 + CW], in_=QT1[0:DH, c0:c0 + CW]
                    )

            def vproj(t):
                vp = po_ps.tile([P, CW], F32, tag="po", name=f"vp{t}")
                for kt in range(NKT):
                    nc.tensor.matmul(
                        vp[:, 0:DQ],
                        lhsT=XV[kt][:, t * P:(t + 1) * P],
                        rhs=WV[:, kt * DQ:(kt + 1) * DQ],
                        start=(kt == 0), stop=(kt == NKT - 1),
                    )
                vv = VP[:, t, :].rearrange("p (h c) -> p h c", c=P)
                nc.vector.tensor_add(
                    vv[:, :, 0:DH],
                    vp[:, 0:DQ].rearrange("p (h d) -> p h d", d=DH),
                    BV[:, :].rearrange("p (h d) -> p h d", d=DH),
                )
                nc.vector.tensor_scalar_mul(
                    vv[:, :, 0:DH], vv[:, :, 0:DH], VM[:, t:t + 1]
                )
                nc.vector.tensor_copy(
                    vv[:, :, DH:DH + 1],
                    VM[:, t:t + 1].to_broadcast([P, HPG, 1]),
                )

            def oproj(m):
                lhA = CTX01[:, m * P:(m + 1) * P]
                lhB = CTX2[:, m * P:(m + 1) * P]
                po_a = po_ps.tile([P, CW], F32, tag="po", name=f"poa{m}")
                nc.tensor.matmul(po_a[:, :], lhsT=lhA, rhs=WO0[:, 0:CW],
                                 start=True, stop=False)
                nc.tensor.matmul(po_a[:, :], lhsT=lhB, rhs=WO2[:, 0:CW],
                                 start=False, stop=True)
                po_b = po_ps.tile([P, CW], F32, tag="po", name=f"pob{m}")
                nc.tensor.matmul(po_b[:, 0:DM - CW], lhsT=lhA, rhs=WO0[:, CW:DM],
                                 start=True, stop=False)
                nc.tensor.matmul(po_b[:, 0:DM - CW], lhsT=lhB, rhs=WO2[:, CW:DM],
                                 start=False, stop=True)
                psb = osb.tile([P, DM], BF, tag="posb", name=f"psb{m}")
                nc.vector.tensor_copy(psb[:, 0:CW], po_a[:, :])
                nc.vector.tensor_copy(psb[:, CW:DM], po_b[:, 0:DM - CW])
                nc.sync.dma_start(out=out[m * P:(m + 1) * P, :], in_=psb)

            # ---- phase A tail: Q projection for chunk 0 ----
            qproj(0, 0)
            qproj(0, 1)

            # ---- attention chunks ----
            def scores01(ci, t):
                sp = sp_ps.tile([P, 2 * CW], F32, tag="sp", name=f"sp{ci}_{t}")
                c0 = ci * CW
                nc.tensor.matmul(
                    sp[:, 0:CW],
                    lhsT=KT0[0:DH, t * P:(t + 1) * P],
                    rhs=QT0[0:DH, c0:c0 + CW],
                    start=True, stop=True,
                )
                nc.tensor.matmul(
                    sp[:, CW:2 * CW],
                    lhsT=KT0[DH:P, t * P:(t + 1) * P],
                    rhs=QT0[DH:P, c0:c0 + CW],
                    start=True, stop=True,
                )
                return sp

            for ci in range(NCH):
                c0 = ci * CW
                ctx0 = ctx_ps.tile([P, CW], F32, tag="ctx", name=f"c0_{ci}")
                ctx1 = ctx_ps.tile([P, CW], F32, tag="ctx", name=f"c1_{ci}")

                def pv01(t, es):
                    nc.tensor.matmul(
                        ctx0[:, :],
                        lhsT=VP[:, t, 0:P],
                        rhs=es[:, 0:CW],
                        start=(t == 0), stop=(t == T - 1),
                    )
                    nc.tensor.matmul(
                        ctx1[:, :],
                        lhsT=VP[:, t, P:2 * P],
                        rhs=es[:, CW:2 * CW],
                        start=(t == 0), stop=(t == T - 1),
                    )

                def do_exp(sp, t):
                    es = espool.tile([P, 2 * CW], BF, tag="es", name=f"es{ci}_{t}")
                    nc.scalar.activation(es, sp, AFT.Exp, bias=0.0, scale=scale)
                    return es

                extras = {}
                if ci == 0:
                    # Deep pipeline: XV lands well after XQ, so scores/exp run
                    # ahead while vproj(t)/pv(t-1) trail one step behind.
                    sps = {0: scores01(ci, 0), 1: scores01(ci, 1)}
                    ess = {0: do_exp(sps.pop(0), 0)}
                    for t in range(T):
                        if t + 2 < T:
                            sps[t + 2] = scores01(ci, t + 2)
                        if t + 1 < T:
                            ess[t + 1] = do_exp(sps.pop(t + 1), t + 1)
                        vproj(t)
                        if t >= 1:
                            pv01(t - 1, ess.pop(t - 1))
                    pv01(T - 1, ess.pop(T - 1))
                else:
                    mlist = [(ci - 1) * 4 + i for i in range(4)]
                    for sl, m in zip([2, 4, 6, 8], mlist):
                        extras[sl] = (lambda mm: (lambda: oproj(mm)))(m)
                    if ci < NCH - 1:
                        nc0 = (ci + 1) * CW
                        extras[3] = (lambda cc: (lambda: qproj(cc, 0)))(nc0)
                        extras[5] = (lambda cc: (lambda: qproj(cc, 1)))(nc0)
                    sp_cur = scores01(ci, 0)
                    for t in range(T):
                        sp_next = scores01(ci, t + 1) if t + 1 < T else None
                        es = do_exp(sp_cur, t)
                        if t + 2 in extras:
                            extras.pop(t + 2)()
                        pv01(t, es)
                        sp_cur = sp_next

                # free ctx0/ctx1: evict numerators + denominators (SBUF), then
                # reciprocals from SBUF. All tensor-tensor inputs share base
                # partition 0 (HW requires equal SB input bases); only outputs
                # may be partition-offset (baseline-proven).
                cs0 = norm.tile([DH, CW], F32, tag="cs0", name=f"cs0_{ci}")
                cs1 = norm.tile([DH, CW], F32, tag="cs1", name=f"cs1_{ci}")
                nc.vector.tensor_copy(cs0, ctx0[0:DH, :])
                nc.vector.tensor_copy(cs1, ctx1[0:DH, :])
                dn = norm.tile([1, 3 * CW], F32, tag="dn", name=f"dn{ci}")
                nc.vector.tensor_copy(dn[:, 0:CW], ctx0[DH:DH + 1, :])
                nc.vector.tensor_copy(dn[:, CW:2 * CW], ctx1[DH:DH + 1, :])
                rc = norm.tile([1, 3 * CW], F32, tag="rc", name=f"rc{ci}")
                nc.vector.reciprocal_approx_fast(rc[:, 0:2 * CW], dn[:, 0:2 * CW])
                bc0 = norm.tile([DH, CW], F32, tag="bc0", name=f"bc0_{ci}")
                bc1 = norm.tile([DH, CW], F32, tag="bc1", name=f"bc1_{ci}")
                nc.gpsimd.partition_broadcast(bc0, rc[:, 0:CW])
                nc.vector.tensor_mul(CTX01[0:DH, c0:c0 + CW], cs0, bc0)
                nc.gpsimd.partition_broadcast(bc1, rc[:, CW:2 * CW])
                nc.vector.tensor_mul(CTX01[DH:P, c0:c0 + CW], cs1, bc1)

                # loop B: head 2, deferred; paired scores via duplicated KT1/QT1
                if ci == 0:
                    ctx2 = ctx_ps.tile([P, CW], F32, tag="ctx", name=f"c2_{ci}")
                else:
                    ctx2 = po_ps.tile([P, CW], F32, tag="po", name=f"c2_{ci}")
                for tg0 in range(0, T, 2):
                    tl = [tg0, tg0 + 1] if tg0 + 1 < T else [tg0]
                    ln = len(tl)
                    sp2 = sp_ps.tile([P, 2 * CW], F32, tag="sp", name=f"sp2_{ci}_{tg0}")
                    for i, t in enumerate(tl):
                        r0 = (i % 2) * DH
                        nc.tensor.matmul(
                            sp2[:, i * CW:(i + 1) * CW],
                            lhsT=KT1[r0:r0 + DH, t * P:(t + 1) * P],
                            rhs=QT1[r0:r0 + DH, c0:c0 + CW],
                            start=True, stop=True,
                        )
                    es2 = espool.tile([P, 2 * CW], BF, tag="es", name=f"es2_{ci}_{tg0}")
                    nc.scalar.activation(
                        es2[:, 0:ln * CW], sp2[:, 0:ln * CW],
                        AFT.Exp, bias=0.0, scale=scale,
                    )
                    if ci == 0 and NCH > 1 and tg0 == 0:
                        qproj(CW, 0)
                    if ci == 0 and NCH > 1 and tg0 == 2:
                        qproj(CW, 1)
                    if tg0 + 2 >= T and extras:
                        for k in sorted(extras):
                            extras.pop(k)()
                    for i, t in enumerate(tl):
                        nc.tensor.matmul(
                            ctx2[:, :],
                            lhsT=VP[:, t, 2 * P:3 * P],
                            rhs=es2[:, i * CW:(i + 1) * CW],
                            start=(t == 0), stop=(t == T - 1),
                        )

                cs2 = norm.tile([DH, CW], F32, tag="cs2", name=f"cs2_{ci}")
                nc.vector.tensor_copy(cs2, ctx2[0:DH, :])
                nc.vector.tensor_copy(dn[:, 2 * CW:3 * CW], ctx2[DH:DH + 1, :])
                nc.vector.reciprocal_approx_fast(rc[:, 2 * CW:3 * CW], dn[:, 2 * CW:3 * CW])
                bc2 = norm.tile([DH, CW], F32, tag="bc2", name=f"bc2_{ci}")
                nc.gpsimd.partition_broadcast(bc2, rc[:, 2 * CW:3 * CW])
                nc.vector.tensor_mul(CTX2[:, c0:c0 + CW], cs2, bc2)

            # tail: O-projection of the last chunk
            for m in range((NCH - 1) * 4, NCH * 4):
                oproj(m)
    nc.compile()
    return nc


def _get_prog(KP):
    if KP not in _prog_cache:
        _prog_cache[KP] = _build_nc(KP)
    return _prog_cache[KP]


def _fmt_w(w):
    # [768, 192] -> [128, 6*192]: row kt*128+p, col j  ->  [p, kt*192+j]
    import ml_dtypes
    return np.ascontiguousarray(
        w.reshape(NKT, P, DQ).transpose(1, 0, 2).reshape(P, NKT * DQ)
    ).astype(ml_dtypes.bfloat16)


def _run(inputs, trace=False):
    import ml_dtypes
    from concourse.bass_utils import run_bass_kernel_spmd

    BF = ml_dtypes.bfloat16

    query = np.asarray(inputs["query"], dtype=np.float32)
    key = np.asarray(inputs["key"], dtype=np.float32)
    value = np.asarray(inputs["value"], dtype=np.float32)
    mask = np.asarray(inputs["mask"])
    Wq = np.asarray(inputs["Wq"], dtype=np.float32)
    bq = np.asarray(inputs["bq"], dtype=np.float32)
    Wk = np.asarray(inputs["Wk"], dtype=np.float32)
    bk = np.asarray(inputs["bk"], dtype=np.float32)
    Wv = np.asarray(inputs["Wv"], dtype=np.float32)
    bv = np.asarray(inputs["bv"], dtype=np.float32)
    Wo = np.asarray(inputs["Wo"], dtype=np.float32)
    bo = np.asarray(inputs["bo"], dtype=np.float32)

    idx = [np.nonzero(mask[b, 0, 0] != 0)[0] for b in range(B)]
    keff = [len(i) for i in idx]
    KP = max(P, ((max(keff) + P - 1) // P) * P)
    T = KP // P

    nc = _get_prog(KP)

    per_batch = {}
    for b in range(B):
        xqT = np.ascontiguousarray(query[b].T).astype(BF)
        xkT = np.zeros((DM, KP), dtype=BF)
        xkT[:, :keff[b]] = key[b][idx[b]].T.astype(BF)
        xvT = np.zeros((DM, KP), dtype=BF)
        xvT[:, :keff[b]] = value[b][idx[b]].T.astype(BF)
        vmf = np.zeros((KP,), dtype=np.float32)
        vmf[:keff[b]] = 1.0
        vm2 = np.ascontiguousarray(vmf.reshape(T, P).T)  # [128, T]
        # packed layouts: one large contiguous DMA each
        xqfm = np.ascontiguousarray(
            xqT.reshape(NKT, P, NCH, CW).transpose(1, 2, 0, 3).reshape(P, -1))
        xkfm = np.ascontiguousarray(
            xkT.reshape(NKT, P, KP).transpose(1, 0, 2).reshape(P, -1))
        xvfm = np.ascontiguousarray(
            xvT.reshape(NKT, P, KP).transpose(1, 0, 2).reshape(P, -1))
        per_batch[b] = (xqfm, xkfm, xvfm, vm2)

    in_maps = []
    for core in range(NCORES):
        b, g = core // G, core % G
        xqfm, xkfm, xvfm, vm2 = per_batch[b]
        sl = slice(g * DQ, (g + 1) * DQ)
        wo_sl = Wo[sl, :]
        wofm = np.zeros((P, 2 * DM), dtype=BF)
        wofm[0:P, 0:DM] = wo_sl[0:P, :].astype(BF)
        wofm[0:DH, DM:2 * DM] = wo_sl[P:DQ, :].astype(BF)
        consm = np.zeros((P, 4 + T + DQ), dtype=np.float32)
        consm[0:P, 0] = bq[sl][0:P]
        consm[0:DH, 1] = bq[sl][P:DQ]
        consm[0:P, 2] = bk[sl][0:P]
        consm[0:DH, 3] = bk[sl][P:DQ]
        consm[:, 4:4 + T] = vm2
        consm[:, 4 + T:4 + T + DQ] = bv[sl].reshape(1, DQ)
        in_maps.append({
            "xqf": xqfm,
            "xkf": xkfm,
            "xvf": xvfm,
            "wq": _fmt_w(Wq[:, sl]),
            "wk": _fmt_w(Wk[:, sl]),
            "wv": _fmt_w(Wv[:, sl]),
            "wof": wofm,
            "cons": consm,
        })

    res = run_bass_kernel_spmd(nc, in_maps, list(range(NCORES)), trace=trace)

    outp = np.zeros((B, S, DM), dtype=np.float32)
    for core in range(NCORES):
        outp[core // G] += res.results[core]["out"].astype(np.float32)
    outp += bo.reshape(1, 1, DM)
    return outp, res


def kernel(**inputs) -> np.ndarray:
    out, _ = _run(inputs, trace=False)
    return out


if __name__ == "__main__":
    nc = _build_nc(1152)
    print("build OK")
